# revision 35
# baseline (speedup 1.0000x reference)
"""Trainium2 Bass kernel for a 2-layer LSTM classifier.

Model:
  x  = embedding[features]            # [B, T, E]
  h1 = LSTM_1(x)      (E=8   -> H=256, TF gate order i,j,f,o, forget bias 1.0)
  h2 = LSTM_2(h1)     (H=256 -> H=256)
  out = h2[:, -1] @ Wd + bd           # [B, V]

B=2048, T=80, V=80, E=8, H=256.

Strategy (data-parallel over batch, 8 cores x 256 rows), fully-skewed
software pipeline so the PE never waits on same-iteration results:

  * Gate-major layout: all on-chip state is [H, B_local]; weights are the
    stationary matmul operand, h streams as the moving operand (bf16,
    1 cyc/row).  Gate columns permuted on host to [f | i | o | j].
  * Iteration t of the emission loop computes: z1[t] h-part (h1[t-1] is one
    iteration old), ALL of z2[t-1] (h1[t-1] and h2[t-2] are old), and the
    onehot part of z1[t+1].  Every matmul therefore depends only on data
    from previous iterations -> PE runs back-to-back at full clock.
  * Layer-1 input path: emb_proj = embedding @ W1[:E] + b1 (+forget bias)
    folded on host into a [V, 4H] table; a one-hot matmul per step
    accumulates it into the same PSUM group as the h-part.
  * Layer-2 bias (b2 + forget bias) via K=1 bias-row matmuls accumulated
    into z2 (exact in bf16 for the b2=0 case).
  * ACT work per step is 7 coarse instructions: sig(f1,i1) [1024] + tanh(j1)
    + sig(o1) + tanh(c1) for layer 1 (the serial recurrence chain, emitted
    first in ACT program order); tanh(j2) + one merged sig(f2,i2,o2) [1536]
    + tanh(c2) for layer 2, scheduled into the chain's gaps.
  * Gates, tanh outputs, and cell state are bf16 in SBUF (DVE 2x mode for
    all cell products/adds).
  * PSUM: z1 in banks 0-3, z2 in banks 4-7; start=True only on each bank's
    first matmul, stop=True on its last (has_written semantics).
"""

import os
import sys

import ml_dtypes
import numpy as np

BF16 = ml_dtypes.bfloat16

for _p in ("/root/.axon_site/_ro/trn_rl_repo", "/opt/trn_rl_repo"):
    if os.path.isdir(_p) and _p not in sys.path:
        sys.path.insert(0, _p)

B, T, V, E, H = 2048, 80, 80, 8, 256
FB = 1.0  # forget-gate bias
NCORES = 8
BL = B // NCORES  # 256 batch rows per core
G4 = 4 * H  # 1024
NM = G4 // 128  # 8 output chunks of 128 gate rows

# on-chip gate order: [f | i | o | j]; chunk m covers gate rows 128m..128m+127
# bank b holds chunks (2b, 2b+1):  f=bank0, i=bank1, o=bank2, j=bank3
_PERM = None


def _perm():
    global _PERM
    if _PERM is None:
        ar = np.arange
        _PERM = np.concatenate(
            [ar(512, 768), ar(0, 256), ar(768, 1024), ar(256, 512)]
        )
    return _PERM


_CACHE = {}


def _build_nc(fast_bias, fb_chunks, n_steps=T):
    """Build the (SPMD, per-core) bass program.

    fast_bias: layer-2 bias handled by the sigmoid(f2) activation bias
    operand (b2+fb uniform across the two f chunks, zero elsewhere).
    fb_chunks: chunk indices needing K=1 bias-row matmuls (generic path).
    """
    import concourse.tile as tile
    from concourse import bacc, mybir

    f32 = mybir.dt.float32
    bf16 = mybir.dt.bfloat16
    AF = mybir.ActivationFunctionType

    nc = bacc.Bacc("TRN2", target_bir_lowering=False, debug=False)

    onehot_d = nc.dram_tensor("onehot", [T, V, BL], bf16, kind="ExternalInput")
    w1h_d = nc.dram_tensor("w1h", [2, 128, G4], bf16, kind="ExternalInput")
    w2x_d = nc.dram_tensor("w2x", [2, 128, G4], bf16, kind="ExternalInput")
    w2h_d = nc.dram_tensor("w2h", [2, 128, G4], bf16, kind="ExternalInput")
    embp_d = nc.dram_tensor("embp", [V, G4], bf16, kind="ExternalInput")
    wd_d = nc.dram_tensor("wd", [2, 128, V], bf16, kind="ExternalInput")
    bdt_d = nc.dram_tensor("bdt", [V, 1], f32, kind="ExternalInput")
    brow_d = nc.dram_tensor("brow", [1, G4], bf16, kind="ExternalInput")
    out_d = nc.dram_tensor("out", [V, BL], f32, kind="ExternalOutput")

    with tile.TileContext(nc) as tc:
        with (
            tc.tile_pool(name="wpool", bufs=1) as wpool,
            tc.tile_pool(name="state", bufs=2) as state,
            tc.tile_pool(name="work", bufs=2) as work,
            tc.tile_pool(name="ohpool", bufs=6) as ohpool,
            tc.tile_pool(name="psum", bufs=1, space="PSUM") as psum,
        ):
            # ---- resident weights ----
            w1h = [wpool.tile([128, G4], bf16, tag=f"w1h{k}", name=f"w1h{k}") for k in range(2)]
            w2x = [wpool.tile([128, G4], bf16, tag=f"w2x{k}", name=f"w2x{k}") for k in range(2)]
            w2h = [wpool.tile([128, G4], bf16, tag=f"w2h{k}", name=f"w2h{k}") for k in range(2)]
            embp = wpool.tile([V, G4], bf16, tag="embp", name="embp")
            wd = [wpool.tile([128, V], bf16, tag=f"wd{k}", name=f"wd{k}") for k in range(2)]
            bdt = wpool.tile([V, 1], f32, tag="bdt", name="bdt")
            for k in range(2):
                nc.sync.dma_start(out=w1h[k][:], in_=w1h_d[k])
                nc.sync.dma_start(out=w2x[k][:], in_=w2x_d[k])
                nc.sync.dma_start(out=w2h[k][:], in_=w2h_d[k])
                nc.sync.dma_start(out=wd[k][:], in_=wd_d[k])
            nc.sync.dma_start(out=embp[:], in_=embp_d[:])
            nc.sync.dma_start(out=bdt[:], in_=bdt_d[:])
            brow = wpool.tile([1, G4], bf16, tag="brow", name="brow")
            ones1 = wpool.tile([1, BL], bf16, tag="ones1", name="ones1")
            nc.sync.dma_start(out=brow[:], in_=brow_d[:])
            nc.gpsimd.memset(ones1[:], 1.0)

            CH = [slice(0, 256), slice(256, 512)]  # h column slices per k-tile
            h1 = c1 = h2 = c2 = None  # state of iteration t-1

            def oh_mms(z, oh, close):
                """onehot part of a z1 accumulation; start=True per bank."""
                for bk in range(4):
                    m0, m1 = 2 * bk, 2 * bk + 1
                    nc.tensor.matmul(z[:, 256 * m0: 256 * (m0 + 1)],
                                     embp[:, 128 * m0: 128 * (m0 + 1)], oh[:],
                                     start=True, stop=False)
                    nc.tensor.matmul(z[:, 256 * m1: 256 * (m1 + 1)],
                                     embp[:, 128 * m1: 128 * (m1 + 1)], oh[:],
                                     start=False, stop=close)

            # ---------------- prologue ----------------
            oh_tiles = {}
            oh_tiles[0] = ohpool.tile([V, BL], bf16, tag="oh", name="oh0")
            nc.sync.dma_start(out=oh_tiles[0][:], in_=onehot_d[0])
            z1 = psum.tile([128, 2048], f32, tag="z1", name="z1_0")
            oh_mms(z1, oh_tiles[0], close=True)  # t=0 has no h-part

            for t in range(n_steps):
                # prefetch next onehot
                if t + 1 < n_steps:
                    ohn = ohpool.tile([V, BL], bf16, tag="oh", name=f"oh{t + 1}")
                    nc.sync.dma_start(out=ohn[:], in_=onehot_d[t + 1])
                    oh_tiles[t + 1] = ohn
                oh_tiles.pop(t - 1, None)

                # ---- PE: z1[t] h-part (banks f,i,j,o: j early for tanh) ----
                if h1 is not None:
                    for bk in (0, 1, 3, 2):
                        m0, m1 = 2 * bk, 2 * bk + 1
                        for i, (m, k) in enumerate(
                            [(m0, 0), (m0, 1), (m1, 0), (m1, 1)]
                        ):
                            nc.tensor.matmul(z1[:, 256 * m: 256 * (m + 1)],
                                             w1h[k][:, 128 * m: 128 * (m + 1)],
                                             h1[:, CH[k]],
                                             start=False, stop=(i == 3))

                # ---- ACT: layer-1 sigmoid f,i (chain head) ----
                g1 = work.tile([128, 1536], bf16, tag="g1", name="g1")
                nc.scalar.activation(g1[:, 0:1024], z1[:, 0:1024], AF.Sigmoid)
                if c1 is not None:
                    ca1 = work.tile([128, 512], bf16, tag="ca1", name="ca1")
                    nc.vector.tensor_mul(ca1[:], c1[:], g1[:, 0:512])

                # ---- PE: all of z2[t-1]; h1-part first, then h2-part ----
                if t >= 1:
                    z2 = psum.tile([128, 2048], f32, tag="z2", name=f"z2_{t - 1}")
                    first = {m: True for m in range(NM)}
                    last_of = {}
                    nmm = 2 if h2 is None else 4
                    for bk in (3, 0, 1, 2):
                        last_of[2 * bk] = last_of[2 * bk + 1] = nmm
                    if not fast_bias:
                        for bk in (3, 0, 1, 2):
                            for m in (2 * bk, 2 * bk + 1):
                                if m in fb_chunks:
                                    nc.tensor.matmul(
                                        z2[:, 256 * m: 256 * (m + 1)],
                                        brow[:, 128 * m: 128 * (m + 1)], ones1[:],
                                        start=first[(m // 2) * 2], stop=False)
                                    first[(m // 2) * 2] = False
                    for part in range(2 if h2 is not None else 1):
                        w2 = w2x if part == 0 else w2h
                        hs = h1 if part == 0 else h2
                        for bk in (3, 0, 1, 2):
                            m0, m1 = 2 * bk, 2 * bk + 1
                            for m, k in [(m0, 0), (m0, 1), (m1, 0), (m1, 1)]:
                                last_of[m] -= 1
                                nc.tensor.matmul(
                                    z2[:, 256 * m: 256 * (m + 1)],
                                    w2[k][:, 128 * m: 128 * (m + 1)], hs[:, CH[k]],
                                    start=first[m0] and m == m0 and k == 0
                                    and part == 0,
                                    stop=(last_of[m] == 0))
                                if m == m0 and k == 0 and part == 0:
                                    first[m0] = False

                # ---- ACT/DVE: finish layer-1 cell of t ----
                tj1 = work.tile([128, 512], bf16, tag="tj1", name="tj1")
                nc.scalar.activation(tj1[:], z1[:, 1536:2048], AF.Tanh)
                c1n = state.tile([128, 512], bf16, tag="c1", name="c1")
                if c1 is None:
                    nc.vector.tensor_mul(c1n[:], g1[:, 512:1024], tj1[:])
                else:
                    t11 = work.tile([128, 512], bf16, tag="t11", name="t11")
                    nc.vector.tensor_mul(t11[:], g1[:, 512:1024], tj1[:])
                    nc.vector.tensor_add(c1n[:], ca1[:], t11[:])

                # layer-1 sigmoid(o): off the critical chain, before thc1
                nc.scalar.activation(g1[:, 1024:1536], z1[:, 1024:1536], AF.Sigmoid)

                # ---- ACT: layer-2 gates of t-1: tanh(j2), then one merged
                # sigmoid(f,i,o) (bias rows already accumulated into z2) ----
                if t >= 1:
                    tj2 = work.tile([128, 512], bf16, tag="tj2", name="tj2")
                    nc.scalar.activation(tj2[:], z2[:, 1536:2048], AF.Tanh)

                thc1 = work.tile([128, 512], bf16, tag="thc1", name="thc1")
                nc.scalar.activation(thc1[:], c1n[:], AF.Tanh)

                if t >= 1:
                    g2 = work.tile([128, 1536], bf16, tag="g2", name="g2")
                    nc.scalar.activation(g2[:], z2[:, 0:1536], AF.Sigmoid)
                    if c2 is not None:
                        ca2 = work.tile([128, 512], bf16, tag="ca2", name="ca2")
                        nc.vector.tensor_mul(ca2[:], c2[:], g2[:, 0:512])

                h1n = state.tile([128, 512], bf16, tag="h1", name="h1")
                nc.vector.tensor_mul(h1n[:], thc1[:], g1[:, 1024:1536])

                if t >= 1:
                    c2n = state.tile([128, 512], bf16, tag="c2", name="c2")
                    if c2 is None:
                        nc.vector.tensor_mul(c2n[:], g2[:, 512:1024], tj2[:])
                    else:
                        t12 = work.tile([128, 512], bf16, tag="t12", name="t12")
                        nc.vector.tensor_mul(t12[:], g2[:, 512:1024], tj2[:])
                        nc.vector.tensor_add(c2n[:], ca2[:], t12[:])
                    thc2 = work.tile([128, 512], bf16, tag="thc2", name="thc2")
                    nc.scalar.activation(thc2[:], c2n[:], AF.Tanh)
                    h2n = state.tile([128, 512], bf16, tag="h2", name="h2")
                    nc.vector.tensor_mul(h2n[:], thc2[:], g2[:, 1024:1536])
                    h2, c2 = h2n, c2n

                # ---- PE: onehot part of z1[t+1] ----
                if t + 1 < n_steps:
                    z1 = psum.tile([128, 2048], f32, tag="z1", name=f"z1_{t + 1}")
                    oh_mms(z1, oh_tiles[t + 1], close=False)

                h1, c1 = h1n, c1n

            # ---------------- epilogue: z2[T-1] + layer-2 cell ----------------
            z2 = psum.tile([128, 2048], f32, tag="z2", name=f"z2_{n_steps - 1}")
            for bk in (3, 0, 1, 2):
                m0, m1 = 2 * bk, 2 * bk + 1
                mms = [(m, w2x[k][:, 128 * m: 128 * (m + 1)], h1[:, CH[k]])
                       for m, k in [(m0, 0), (m0, 1), (m1, 0), (m1, 1)]]
                if h2 is not None:
                    mms += [(m, w2h[k][:, 128 * m: 128 * (m + 1)], h2[:, CH[k]])
                            for m, k in [(m0, 0), (m0, 1), (m1, 0), (m1, 1)]]
                if not fast_bias:
                    mms += [(m, brow[:, 128 * m: 128 * (m + 1)], ones1[:])
                            for m in (m0, m1) if m in fb_chunks]
                for i, (m, lhsT, rhs) in enumerate(mms):
                    nc.tensor.matmul(z2[:, 256 * m: 256 * (m + 1)], lhsT, rhs,
                                     start=(i == 0), stop=(i == len(mms) - 1))
            tj2 = work.tile([128, 512], bf16, tag="tj2", name="tj2_e")
            nc.scalar.activation(tj2[:], z2[:, 1536:2048], AF.Tanh)
            g2 = work.tile([128, 1536], bf16, tag="g2", name="g2_e")
            nc.scalar.activation(g2[:], z2[:, 0:1536], AF.Sigmoid)
            ca2 = work.tile([128, 512], bf16, tag="ca2", name="ca2_e")
            nc.vector.tensor_mul(ca2[:], c2[:], g2[:, 0:512])
            t12 = work.tile([128, 512], bf16, tag="t12", name="t12_e")
            nc.vector.tensor_mul(t12[:], g2[:, 512:1024], tj2[:])
            c2n = state.tile([128, 512], bf16, tag="c2", name="c2_e")
            nc.vector.tensor_add(c2n[:], ca2[:], t12[:])
            thc2 = work.tile([128, 512], bf16, tag="thc2", name="thc2_e")
            nc.scalar.activation(thc2[:], c2n[:], AF.Tanh)
            h2n = state.tile([128, 512], bf16, tag="h2", name="h2_e")
            nc.vector.tensor_mul(h2n[:], thc2[:], g2[:, 1024:1536])

            # ---------------- dense head on final h2 ----------------
            lg = psum.tile([128, 2048], f32, tag="z1", name="lg")
            nc.tensor.matmul(lg[0:V, 0:BL], wd[0][:], h2n[:, CH[0]],
                             start=True, stop=False)
            nc.tensor.matmul(lg[0:V, 0:BL], wd[1][:], h2n[:, CH[1]],
                             start=False, stop=True)
            outs = work.tile([V, BL], f32, tag="outs", name="outs")
            nc.scalar.add(outs[:], lg[0:V, 0:BL], bdt[:])
            nc.sync.dma_start(out=out_d[:], in_=outs[:])

    nc.compile()
    return nc


def _get_nc(key):
    fast_bias, fb_chunks = key
    ck = ("nc", key)
    if ck not in _CACHE:
        _CACHE[ck] = _build_nc(fast_bias, fb_chunks)
    return _CACHE[ck]


def _prep_inputs(features, embedding, W1, b1, W2, b2, Wd, bd):
    """Host-side weight folding / layout prep -> (per-core input maps, key)."""
    features = np.asarray(features, np.int32)
    embedding = np.asarray(embedding, np.float32)
    W1 = np.asarray(W1, np.float32)
    b1 = np.asarray(b1, np.float32)
    W2 = np.asarray(W2, np.float32)
    b2 = np.asarray(b2, np.float32)
    Wd = np.asarray(Wd, np.float32)
    bd = np.asarray(bd, np.float32)

    p = _perm()
    W1p = W1[:, p]
    W2p = W2[:, p]
    b1p = b1[p]
    b2p = b2[p]
    fbvec = np.zeros(G4, np.float32)
    fbvec[0:256] = FB  # f block sits first in permuted order

    embp = (embedding @ W1p[:E] + (b1p + fbvec)).astype(BF16)  # [V, 4H]
    w1h = np.ascontiguousarray(W1p[E:].reshape(2, 128, G4).astype(BF16))
    w2x = np.ascontiguousarray(W2p[:H].reshape(2, 128, G4).astype(BF16))
    w2h = np.ascontiguousarray(W2p[H:].reshape(2, 128, G4).astype(BF16))
    wd = np.ascontiguousarray(Wd.reshape(2, 128, V).astype(BF16))
    bdt = np.ascontiguousarray(bd.reshape(V, 1))

    b2full = (b2p + fbvec).astype(np.float32)
    fast_bias = False  # bias rows via K=1 matmuls; enables merged sigmoid(f,i,o)
    fb_chunks = tuple(
        m for m in range(NM)
        if np.any(b2full[128 * m: 128 * (m + 1)] != 0.0)
    )
    shared = {
        "w1h": w1h, "w2x": w2x, "w2h": w2h, "embp": embp,
        "wd": wd, "bdt": bdt,
        "brow": np.ascontiguousarray(b2full.reshape(1, G4).astype(BF16)),
    }

    eye = np.eye(V, dtype=BF16)
    in_maps = []
    for c in range(NCORES):
        f = features[c * BL: (c + 1) * BL]  # [BL, T]
        oh = eye[f.T]  # [T, BL, V]
        oh = np.ascontiguousarray(oh.transpose(0, 2, 1))  # [T, V, BL]
        m = dict(shared)
        m["onehot"] = oh
        in_maps.append(m)
    return in_maps, (fast_bias, fb_chunks)


def _run(in_maps, key, trace=False):
    from concourse.bass_utils import run_bass_kernel_spmd

    nc = _get_nc(key)
    res = run_bass_kernel_spmd(nc, in_maps, list(range(NCORES)), trace=trace)
    logits = np.concatenate([r["out"].T for r in res.results], axis=0)  # [B, V]
    return logits.astype(np.float32), res


def kernel(features, embedding, W1, b1, W2, b2, Wd, bd):
    in_maps, key = _prep_inputs(features, embedding, W1, b1, W2, b2, Wd, bd)
    logits, _ = _run(in_maps, key, trace=False)
    return logits


# revision 41
# speedup vs baseline: 1.0091x; 1.0091x over previous
"""Trainium2 Bass kernel for a 2-layer LSTM classifier.

Model:
  x  = embedding[features]            # [B, T, E]
  h1 = LSTM_1(x)      (E=8   -> H=256, TF gate order i,j,f,o, forget bias 1.0)
  h2 = LSTM_2(h1)     (H=256 -> H=256)
  out = h2[:, -1] @ Wd + bd           # [B, V]

B=2048, T=80, V=80, E=8, H=256.

Strategy (data-parallel over batch, 8 cores x 256 rows), fully-skewed
software pipeline so the PE never waits on same-iteration results:

  * Gate-major layout: all on-chip state is [H, B_local]; weights are the
    stationary matmul operand, h streams as the moving operand (bf16,
    1 cyc/row).  Gate columns permuted on host to [f | i | o | j].
  * Iteration t of the emission loop computes: z1[t] h-part (h1[t-1] is one
    iteration old), ALL of z2[t-1] (h1[t-1] and h2[t-2] are old), and the
    onehot part of z1[t+1].  Every matmul therefore depends only on data
    from previous iterations -> PE runs back-to-back at full clock.
  * Layer-1 input path: emb_proj = embedding @ W1[:E] + b1 (+forget bias)
    folded on host into a [V, 4H] table; a one-hot matmul per step
    accumulates it into the same PSUM group as the h-part.
  * Layer-2 bias (b2 + forget bias) via K=1 bias-row matmuls accumulated
    into z2 (exact in bf16 for the b2=0 case).
  * ACT work per step is 7 coarse instructions: sig(f1,i1) [1024] + tanh(j1)
    + sig(o1) + tanh(c1) for layer 1 (the serial recurrence chain, emitted
    first in ACT program order); tanh(j2) + one merged sig(f2,i2,o2) [1536]
    + tanh(c2) for layer 2, scheduled into the chain's gaps.
  * Gates, tanh outputs, and cell state are bf16 in SBUF (DVE 2x mode for
    all cell products/adds).
  * PSUM: z1 in banks 0-3, z2 in banks 4-7; start=True only on each bank's
    first matmul, stop=True on its last (has_written semantics).
"""

import os
import sys

import ml_dtypes
import numpy as np

BF16 = ml_dtypes.bfloat16

for _p in ("/root/.axon_site/_ro/trn_rl_repo", "/opt/trn_rl_repo"):
    if os.path.isdir(_p) and _p not in sys.path:
        sys.path.insert(0, _p)

B, T, V, E, H = 2048, 80, 80, 8, 256
FB = 1.0  # forget-gate bias
NCORES = 8
BL = B // NCORES  # 256 batch rows per core
G4 = 4 * H  # 1024
NM = G4 // 128  # 8 output chunks of 128 gate rows

# on-chip gate order: [f | i | o | j]; chunk m covers gate rows 128m..128m+127
# bank b holds chunks (2b, 2b+1):  f=bank0, i=bank1, o=bank2, j=bank3
_PERM = None


def _perm():
    global _PERM
    if _PERM is None:
        ar = np.arange
        _PERM = np.concatenate(
            [ar(512, 768), ar(0, 256), ar(768, 1024), ar(256, 512)]
        )
    return _PERM


_CACHE = {}


def _build_nc(fast_bias, fb_chunks, n_steps=T):
    """Build the (SPMD, per-core) bass program.

    fast_bias: layer-2 bias handled by the sigmoid(f2) activation bias
    operand (b2+fb uniform across the two f chunks, zero elsewhere).
    fb_chunks: chunk indices needing K=1 bias-row matmuls (generic path).
    """
    import concourse.tile as tile
    from concourse import bacc, mybir

    f32 = mybir.dt.float32
    bf16 = mybir.dt.bfloat16
    AF = mybir.ActivationFunctionType

    nc = bacc.Bacc("TRN2", target_bir_lowering=False, debug=False)

    onehot_d = nc.dram_tensor("onehot", [T, V, BL], bf16, kind="ExternalInput")
    w1h_d = nc.dram_tensor("w1h", [2, 128, G4], bf16, kind="ExternalInput")
    w2x_d = nc.dram_tensor("w2x", [2, 128, G4], bf16, kind="ExternalInput")
    w2h_d = nc.dram_tensor("w2h", [2, 128, G4], bf16, kind="ExternalInput")
    embp_d = nc.dram_tensor("embp", [V, G4], bf16, kind="ExternalInput")
    wd_d = nc.dram_tensor("wd", [2, 128, V], bf16, kind="ExternalInput")
    bdt_d = nc.dram_tensor("bdt", [V, 1], f32, kind="ExternalInput")
    brow_d = nc.dram_tensor("brow", [1, G4], bf16, kind="ExternalInput")
    out_d = nc.dram_tensor("out", [V, BL], f32, kind="ExternalOutput")

    with tile.TileContext(nc) as tc:
        with (
            tc.tile_pool(name="wpool", bufs=1) as wpool,
            tc.tile_pool(name="state", bufs=2) as state,
            tc.tile_pool(name="work", bufs=2) as work,
            tc.tile_pool(name="ohpool", bufs=6) as ohpool,
            tc.tile_pool(name="psum", bufs=1, space="PSUM") as psum,
        ):
            # ---- resident weights ----
            w1h = [wpool.tile([128, G4], bf16, tag=f"w1h{k}", name=f"w1h{k}") for k in range(2)]
            w2x = [wpool.tile([128, G4], bf16, tag=f"w2x{k}", name=f"w2x{k}") for k in range(2)]
            w2h = [wpool.tile([128, G4], bf16, tag=f"w2h{k}", name=f"w2h{k}") for k in range(2)]
            embp = wpool.tile([V, G4], bf16, tag="embp", name="embp")
            wd = [wpool.tile([128, V], bf16, tag=f"wd{k}", name=f"wd{k}") for k in range(2)]
            bdt = wpool.tile([V, 1], f32, tag="bdt", name="bdt")
            # oh[0] and embp gate the first onehot matmuls; fetch them ahead
            # of the bulk weights
            oh_tiles = {}
            oh_tiles[0] = ohpool.tile([V, BL], bf16, tag="oh", name="oh0")
            nc.sync.dma_start(out=oh_tiles[0][:], in_=onehot_d[0])
            nc.sync.dma_start(out=embp[:], in_=embp_d[:])
            for k in range(2):
                nc.sync.dma_start(out=w1h[k][:], in_=w1h_d[k])
            for k in range(2):
                nc.sync.dma_start(out=w2x[k][:], in_=w2x_d[k])
                nc.sync.dma_start(out=w2h[k][:], in_=w2h_d[k])
                nc.sync.dma_start(out=wd[k][:], in_=wd_d[k])
            nc.sync.dma_start(out=bdt[:], in_=bdt_d[:])
            brow = wpool.tile([1, G4], bf16, tag="brow", name="brow")
            ones1 = wpool.tile([1, BL], bf16, tag="ones1", name="ones1")
            nc.sync.dma_start(out=brow[:], in_=brow_d[:])
            nc.gpsimd.memset(ones1[:], 1.0)

            CH = [slice(0, 256), slice(256, 512)]  # h column slices per k-tile
            h1 = c1 = h2 = c2 = None  # state of iteration t-1

            def oh_mms(z, oh, close):
                """onehot part of a z1 accumulation; start=True per bank."""
                for bk in range(4):
                    m0, m1 = 2 * bk, 2 * bk + 1
                    nc.tensor.matmul(z[:, 256 * m0: 256 * (m0 + 1)],
                                     embp[:, 128 * m0: 128 * (m0 + 1)], oh[:],
                                     start=True, stop=False)
                    nc.tensor.matmul(z[:, 256 * m1: 256 * (m1 + 1)],
                                     embp[:, 128 * m1: 128 * (m1 + 1)], oh[:],
                                     start=False, stop=close)

            # ---------------- prologue ----------------
            z1 = psum.tile([128, 2048], f32, tag="z1", name="z1_0")
            oh_mms(z1, oh_tiles[0], close=True)  # t=0 has no h-part

            for t in range(n_steps):
                # prefetch next onehot
                if t + 1 < n_steps and t + 1 not in oh_tiles:
                    ohn = ohpool.tile([V, BL], bf16, tag="oh", name=f"oh{t + 1}")
                    nc.sync.dma_start(out=ohn[:], in_=onehot_d[t + 1])
                    oh_tiles[t + 1] = ohn
                oh_tiles.pop(t - 1, None)

                # ---- PE: z1[t] h-part (banks f,i,j,o: j early for tanh) ----
                if h1 is not None:
                    for bk in (0, 1, 3, 2):
                        m0, m1 = 2 * bk, 2 * bk + 1
                        for i, (m, k) in enumerate(
                            [(m0, 0), (m0, 1), (m1, 0), (m1, 1)]
                        ):
                            nc.tensor.matmul(z1[:, 256 * m: 256 * (m + 1)],
                                             w1h[k][:, 128 * m: 128 * (m + 1)],
                                             h1[:, CH[k]],
                                             start=False, stop=(i == 3))

                # ---- ACT: layer-1 sigmoid f,i (chain head) ----
                g1 = work.tile([128, 1536], bf16, tag="g1", name="g1")
                nc.scalar.activation(g1[:, 0:1024], z1[:, 0:1024], AF.Sigmoid)
                if c1 is not None:
                    ca1 = work.tile([128, 512], bf16, tag="ca1", name="ca1")
                    nc.vector.tensor_mul(ca1[:], c1[:], g1[:, 0:512])

                # ---- PE: all of z2[t-1]; h1-part first, then h2-part ----
                if t >= 1:
                    z2 = psum.tile([128, 2048], f32, tag="z2", name=f"z2_{t - 1}")
                    first = {m: True for m in range(NM)}
                    last_of = {}
                    nmm = 2 if h2 is None else 4
                    for bk in (3, 0, 1, 2):
                        last_of[2 * bk] = last_of[2 * bk + 1] = nmm
                    if not fast_bias:
                        for bk in (3, 0, 1, 2):
                            for m in (2 * bk, 2 * bk + 1):
                                if m in fb_chunks:
                                    nc.tensor.matmul(
                                        z2[:, 256 * m: 256 * (m + 1)],
                                        brow[:, 128 * m: 128 * (m + 1)], ones1[:],
                                        start=first[(m // 2) * 2], stop=False)
                                    first[(m // 2) * 2] = False
                    for part in range(2 if h2 is not None else 1):
                        w2 = w2x if part == 0 else w2h
                        hs = h1 if part == 0 else h2
                        for bk in (3, 0, 1, 2):
                            m0, m1 = 2 * bk, 2 * bk + 1
                            for m, k in [(m0, 0), (m0, 1), (m1, 0), (m1, 1)]:
                                last_of[m] -= 1
                                nc.tensor.matmul(
                                    z2[:, 256 * m: 256 * (m + 1)],
                                    w2[k][:, 128 * m: 128 * (m + 1)], hs[:, CH[k]],
                                    start=first[m0] and m == m0 and k == 0
                                    and part == 0,
                                    stop=(last_of[m] == 0))
                                if m == m0 and k == 0 and part == 0:
                                    first[m0] = False

                # ---- ACT/DVE: finish layer-1 cell of t ----
                tj1 = work.tile([128, 512], bf16, tag="tj1", name="tj1")
                nc.scalar.activation(tj1[:], z1[:, 1536:2048], AF.Tanh)
                c1n = state.tile([128, 512], bf16, tag="c1", name="c1")
                if c1 is None:
                    nc.vector.tensor_mul(c1n[:], g1[:, 512:1024], tj1[:])
                else:
                    t11 = work.tile([128, 512], bf16, tag="t11", name="t11")
                    nc.vector.tensor_mul(t11[:], g1[:, 512:1024], tj1[:])
                    nc.vector.tensor_add(c1n[:], ca1[:], t11[:])

                # layer-1 sigmoid(o): off the critical chain, before thc1
                nc.scalar.activation(g1[:, 1024:1536], z1[:, 1024:1536], AF.Sigmoid)

                # ---- ACT: layer-2 gates of t-1: tanh(j2), then one merged
                # sigmoid(f,i,o) (bias rows already accumulated into z2) ----
                if t >= 1:
                    tj2 = work.tile([128, 512], bf16, tag="tj2", name="tj2")
                    nc.scalar.activation(tj2[:], z2[:, 1536:2048], AF.Tanh)

                thc1 = work.tile([128, 512], bf16, tag="thc1", name="thc1")
                nc.scalar.activation(thc1[:], c1n[:], AF.Tanh)

                if t >= 1:
                    g2 = work.tile([128, 1536], bf16, tag="g2", name="g2")
                    nc.scalar.activation(g2[:], z2[:, 0:1536], AF.Sigmoid)
                    if c2 is not None:
                        ca2 = work.tile([128, 512], bf16, tag="ca2", name="ca2")
                        nc.vector.tensor_mul(ca2[:], c2[:], g2[:, 0:512])

                h1n = state.tile([128, 512], bf16, tag="h1", name="h1")
                nc.vector.tensor_mul(h1n[:], thc1[:], g1[:, 1024:1536])

                if t >= 1:
                    c2n = state.tile([128, 512], bf16, tag="c2", name="c2")
                    if c2 is None:
                        nc.vector.tensor_mul(c2n[:], g2[:, 512:1024], tj2[:])
                    else:
                        t12 = work.tile([128, 512], bf16, tag="t12", name="t12")
                        nc.vector.tensor_mul(t12[:], g2[:, 512:1024], tj2[:])
                        nc.vector.tensor_add(c2n[:], ca2[:], t12[:])
                    thc2 = work.tile([128, 512], bf16, tag="thc2", name="thc2")
                    nc.scalar.activation(thc2[:], c2n[:], AF.Tanh)
                    h2n = state.tile([128, 512], bf16, tag="h2", name="h2")
                    nc.vector.tensor_mul(h2n[:], thc2[:], g2[:, 1024:1536])
                    h2, c2 = h2n, c2n

                # ---- PE: onehot part of z1[t+1] ----
                if t + 1 < n_steps:
                    z1 = psum.tile([128, 2048], f32, tag="z1", name=f"z1_{t + 1}")
                    oh_mms(z1, oh_tiles[t + 1], close=False)

                h1, c1 = h1n, c1n

            # ---------------- epilogue: z2[T-1] + layer-2 cell ----------------
            z2 = psum.tile([128, 2048], f32, tag="z2", name=f"z2_{n_steps - 1}")
            for bk in (3, 0, 1, 2):
                m0, m1 = 2 * bk, 2 * bk + 1
                mms = [(m, w2x[k][:, 128 * m: 128 * (m + 1)], h1[:, CH[k]])
                       for m, k in [(m0, 0), (m0, 1), (m1, 0), (m1, 1)]]
                if h2 is not None:
                    mms += [(m, w2h[k][:, 128 * m: 128 * (m + 1)], h2[:, CH[k]])
                            for m, k in [(m0, 0), (m0, 1), (m1, 0), (m1, 1)]]
                if not fast_bias:
                    mms += [(m, brow[:, 128 * m: 128 * (m + 1)], ones1[:])
                            for m in (m0, m1) if m in fb_chunks]
                for i, (m, lhsT, rhs) in enumerate(mms):
                    nc.tensor.matmul(z2[:, 256 * m: 256 * (m + 1)], lhsT, rhs,
                                     start=(i == 0), stop=(i == len(mms) - 1))
            # per-bank gate activations so each starts as its bank lands
            tj2 = work.tile([128, 512], bf16, tag="tj2", name="tj2_e")
            nc.scalar.activation(tj2[:], z2[:, 1536:2048], AF.Tanh)
            g2 = work.tile([128, 1536], bf16, tag="g2", name="g2_e")
            nc.scalar.activation(g2[:, 0:512], z2[:, 0:512], AF.Sigmoid)
            nc.scalar.activation(g2[:, 512:1024], z2[:, 512:1024], AF.Sigmoid)
            ca2 = work.tile([128, 512], bf16, tag="ca2", name="ca2_e")
            nc.vector.tensor_mul(ca2[:], c2[:], g2[:, 0:512])
            t12 = work.tile([128, 512], bf16, tag="t12", name="t12_e")
            nc.vector.tensor_mul(t12[:], g2[:, 512:1024], tj2[:])
            c2n = state.tile([128, 512], bf16, tag="c2", name="c2_e")
            nc.vector.tensor_add(c2n[:], ca2[:], t12[:])
            nc.scalar.activation(g2[:, 1024:1536], z2[:, 1024:1536], AF.Sigmoid)
            thc2 = work.tile([128, 512], bf16, tag="thc2", name="thc2_e")
            nc.scalar.activation(thc2[:], c2n[:], AF.Tanh)
            h2n = state.tile([128, 512], bf16, tag="h2", name="h2_e")
            nc.vector.tensor_mul(h2n[:], thc2[:], g2[:, 1024:1536])

            # ---------------- dense head on final h2 ----------------
            lg = psum.tile([128, 2048], f32, tag="z1", name="lg")
            nc.tensor.matmul(lg[0:V, 0:BL], wd[0][:], h2n[:, CH[0]],
                             start=True, stop=False)
            nc.tensor.matmul(lg[0:V, 0:BL], wd[1][:], h2n[:, CH[1]],
                             start=False, stop=True)
            outs = work.tile([V, BL], f32, tag="outs", name="outs")
            nc.scalar.add(outs[:], lg[0:V, 0:BL], bdt[:])
            nc.sync.dma_start(out=out_d[:], in_=outs[:])

    nc.compile()
    return nc


def _get_nc(key):
    fast_bias, fb_chunks = key
    ck = ("nc", key)
    if ck not in _CACHE:
        _CACHE[ck] = _build_nc(fast_bias, fb_chunks)
    return _CACHE[ck]


def _prep_inputs(features, embedding, W1, b1, W2, b2, Wd, bd):
    """Host-side weight folding / layout prep -> (per-core input maps, key)."""
    features = np.asarray(features, np.int32)
    embedding = np.asarray(embedding, np.float32)
    W1 = np.asarray(W1, np.float32)
    b1 = np.asarray(b1, np.float32)
    W2 = np.asarray(W2, np.float32)
    b2 = np.asarray(b2, np.float32)
    Wd = np.asarray(Wd, np.float32)
    bd = np.asarray(bd, np.float32)

    p = _perm()
    W1p = W1[:, p]
    W2p = W2[:, p]
    b1p = b1[p]
    b2p = b2[p]
    fbvec = np.zeros(G4, np.float32)
    fbvec[0:256] = FB  # f block sits first in permuted order

    embp = (embedding @ W1p[:E] + (b1p + fbvec)).astype(BF16)  # [V, 4H]
    w1h = np.ascontiguousarray(W1p[E:].reshape(2, 128, G4).astype(BF16))
    w2x = np.ascontiguousarray(W2p[:H].reshape(2, 128, G4).astype(BF16))
    w2h = np.ascontiguousarray(W2p[H:].reshape(2, 128, G4).astype(BF16))
    wd = np.ascontiguousarray(Wd.reshape(2, 128, V).astype(BF16))
    bdt = np.ascontiguousarray(bd.reshape(V, 1))

    b2full = (b2p + fbvec).astype(np.float32)
    fast_bias = False  # bias rows via K=1 matmuls; enables merged sigmoid(f,i,o)
    fb_chunks = tuple(
        m for m in range(NM)
        if np.any(b2full[128 * m: 128 * (m + 1)] != 0.0)
    )
    shared = {
        "w1h": w1h, "w2x": w2x, "w2h": w2h, "embp": embp,
        "wd": wd, "bdt": bdt,
        "brow": np.ascontiguousarray(b2full.reshape(1, G4).astype(BF16)),
    }

    eye = np.eye(V, dtype=BF16)
    in_maps = []
    for c in range(NCORES):
        f = features[c * BL: (c + 1) * BL]  # [BL, T]
        oh = eye[f.T]  # [T, BL, V]
        oh = np.ascontiguousarray(oh.transpose(0, 2, 1))  # [T, V, BL]
        m = dict(shared)
        m["onehot"] = oh
        in_maps.append(m)
    return in_maps, (fast_bias, fb_chunks)


def _run(in_maps, key, trace=False):
    from concourse.bass_utils import run_bass_kernel_spmd

    nc = _get_nc(key)
    res = run_bass_kernel_spmd(nc, in_maps, list(range(NCORES)), trace=trace)
    logits = np.concatenate([r["out"].T for r in res.results], axis=0)  # [B, V]
    return logits.astype(np.float32), res


def kernel(features, embedding, W1, b1, W2, b2, Wd, bd):
    in_maps, key = _prep_inputs(features, embedding, W1, b1, W2, b2, Wd, bd)
    logits, _ = _run(in_maps, key, trace=False)
    return logits
